# revision 16
# baseline (speedup 1.0000x reference)
"""Trainium2 Bass kernel: weighted BCE + IoU loss (structure loss).

Full inputs: pred/mask [64, 1, 512, 512] fp32.  Data-parallel over 8
NeuronCores (8 images per core).  Each core computes, per image,
  swt = sum((a + 0.2) * t)   and   sa = sum(a)
where
  a = |boxsum31x31(mask)/961 - mask|            (weight = 1 + 5a)
  t = ln(1+E) - P*M + num/den,   E = e^P,
  num = M + (1-M)E,  den = 1 + M + 2E     (= bce + iou of the reference)
Host finishes:  loss_img = 5*swt / (512*512 + 5*sa),  output = mean.

Implementation notes:
- Host passes bf16 pred/mask plus two affine mask variants mh=(M+1)/2 and
  hm2=(M-1)/2.  The pointwise chain then runs as 2x-mode bf16
  TensorTensor ops:
    den/2 = (E + mh) + 0.5          -> +0.5 folds into the Ln bias
    num/2 = (mh - hm2*E) - 0.5      -> -0.5 folds into the Ln bias
    num/den = exp(ln(num/2) - ln(den/2))   (ln2 cancels)
- 31x31 box filter = two banded {0,1}-matmuls on the tensor engine with
  transposes (DMA xbar or PE) between passes; x = T2/961 - M is fused
  into the PSUM read (scalar_tensor_tensor).
- All activation funcs (Exp/Ln/Abs/Copy) are pinned to one ACT table set.
- Image sums ride on accum_out of ops we already run; a final ones-matmul
  reduces partitions.
"""

import os as _os
from contextlib import ExitStack

import numpy as np

_B = 64
_H = 512
_W = 512
_NC = 8
_BPC = _B // _NC  # images per core
_HW = float(_H * _W)
_KHALF = 15  # box filter half width (31 taps)

_CACHE = {}

# tuning toggles (resolved at build time; set K_* env vars to override)
_X_FUSED = _os.environ.get("K_X_FUSED", "1") == "1"
_DMA_T_MT = _os.environ.get("K_DMA_T_MT", "1") == "1"  # mtb via DMA xbar (else PE)
_DMA_T_UT = _os.environ.get("K_DMA_T_UT", "0") == "1"  # utb via DMA xbar (else PE)
_GP_DMA = _os.environ.get("K_GP_DMA", "0") == "1"      # pb/hm2 loads via gpsimd
_MBUFS = int(_os.environ.get("K_MBUFS", "2"))
_INPLACE = _os.environ.get("K_INPLACE", "0") == "1"
_IBUFS = int(_os.environ.get("K_IBUFS", "2"))
_ABS_ACT = _os.environ.get("K_ABS_ACT", "1") == "1"    # |x| on ACT (else DVE)
_HOST_MT = _os.environ.get("K_HOST_MT", "1") == "1"    # maskT as host input
_HM2_POOL = _os.environ.get("K_HM2_POOL", "0") == "1"  # hm2 = mh-1 on gpsimd (drop input)
_XWIDE = _os.environ.get("K_XWIDE", "0") == "1"        # T2 as one [128,2048] psum tile
_GP_MT = _os.environ.get("K_GP_MT", "0") == "1"        # maskt load via gpsimd


def _band_np():
    import ml_dtypes

    idx = np.arange(_H)
    b = (np.abs(idx[:, None] - idx[None, :]) <= _KHALF).astype(np.float32)
    return b.astype(ml_dtypes.bfloat16)


def _pin_act_table_set():
    """Keep every activation in natural_log_exp_and_others (has Exp, Ln,
    Abs, Copy, Identity) so the kernel needs exactly one ACT table load."""
    import concourse.bacc as bacc_mod
    import concourse.bass_interp as interp_mod
    from concourse.hw_specs import get_activation_tables as real_gat

    keep = "natural_log_exp_and_others"

    def patched(arch):
        t = real_gat(arch)
        return {k: (v if k == keep else set()) for k, v in t.items()}

    bacc_mod.get_activation_tables = patched
    interp_mod.get_activation_tables = patched


def _build():
    if "nc" in _CACHE:
        return _CACHE["nc"]

    import concourse.bass as bass
    import concourse.tile as tile
    from concourse import bacc, mybir

    _pin_act_table_set()

    AF = mybir.ActivationFunctionType
    ALU = mybir.AluOpType
    F32 = mybir.dt.float32
    BF16 = mybir.dt.bfloat16
    ts = bass.ts

    nc = bacc.Bacc(
        "TRN2", target_bir_lowering=False, debug=False, num_devices=_NC
    )
    # register the Ln-bias constants (0.0/1.0 are preregistered by Bass)
    for val in (0.5, -0.5):
        t = nc.alloc_sbuf_tensor(f"const-f32-{val}", [128, 1], F32)
        nc.gpsimd.memset(t.ap(), val)
        nc.const_aps.aps[(F32, val)] = t.ap()
    nc.all_engine_barrier()

    pred_d = nc.dram_tensor("pred", [_BPC, _H, _W], BF16, kind="ExternalInput").ap()
    mask_d = nc.dram_tensor("mask", [_BPC, _H, _W], BF16, kind="ExternalInput").ap()
    mh_d = nc.dram_tensor("mh", [_BPC, _H, _W], BF16, kind="ExternalInput").ap()
    hm2_d = (
        None
        if _HM2_POOL
        else nc.dram_tensor("hm2", [_BPC, _H, _W], BF16, kind="ExternalInput").ap()
    )
    maskt_d = (
        nc.dram_tensor("maskt", [_BPC, _W, _H], BF16, kind="ExternalInput").ap()
        if _HOST_MT
        else None
    )
    band_d = nc.dram_tensor("band", [_H, _W], BF16, kind="ExternalInput").ap()
    ident_d = nc.dram_tensor("ident", [128, 128], BF16, kind="ExternalInput").ap()
    ones_d = nc.dram_tensor("ones", [128, 1], F32, kind="ExternalInput").ap()
    out_d = nc.dram_tensor("out", [1, 2 * _BPC], F32, kind="ExternalOutput").ap()

    with tile.TileContext(nc) as tc, ExitStack() as ctx:
        cpool = ctx.enter_context(tc.tile_pool(name="cpool", bufs=1))
        ipool = ctx.enter_context(tc.tile_pool(name="ipool", bufs=_IBUFS))
        mpool = ctx.enter_context(tc.tile_pool(name="mpool", bufs=_MBUFS))
        pup = ctx.enter_context(tc.tile_pool(name="pup", bufs=2, space="PSUM"))
        ptp = ctx.enter_context(
            tc.tile_pool(name="ptp", bufs=(1 if _XWIDE else 2), space="PSUM")
        )
        put = ctx.enter_context(
            tc.tile_pool(name="put", bufs=(1 if _XWIDE else 2), space="PSUM"))
        pfin = ctx.enter_context(tc.tile_pool(name="pfin", bufs=1, space="PSUM"))

        band_sb = cpool.tile([128, 4, _W], BF16, name="band_sb", tag="band_sb")
        nc.sync.dma_start(band_sb[:], band_d.rearrange("(j p) c -> p j c", p=128))
        ident_sb = cpool.tile([128, 128], BF16, name="ident_sb", tag="ident_sb")
        nc.sync.dma_start(ident_sb[:], ident_d)
        ones_sb = cpool.tile([128, 1], F32, name="ones_sb", tag="ones_sb")
        nc.sync.dma_start(ones_sb[:], ones_d)
        # per-partition accumulators: col 2i = sum((a+0.2)t), col 2i+1 = sum(a)
        acc = cpool.tile([128, 2 * _BPC], F32, name="acc", tag="acc")
        one4 = None
        if _HM2_POOL:
            one4 = cpool.tile([128, 4, _W], BF16, name="one4", tag="one4")
            nc.gpsimd.memset(one4[:], 1.0)

        for i in range(_BPC):
            # ---------------- loads ----------------
            dmae = nc.gpsimd if _GP_DMA else nc.sync
            pb = ipool.tile([128, 4, _W], BF16, name="pb", tag="pb")
            dmae.dma_start(pb[:], pred_d[i].rearrange("(j p) w -> p j w", p=128))
            mb = ipool.tile([128, 4, _W], BF16, name="mb", tag="mb")
            nc.sync.dma_start(mb[:], mask_d[i].rearrange("(j p) w -> p j w", p=128))
            mh = ipool.tile([128, 4, _W], BF16, name="mh", tag="mh")
            nc.sync.dma_start(mh[:], mh_d[i].rearrange("(j p) w -> p j w", p=128))
            hm2 = ipool.tile([128, 4, _W], BF16, name="hm2", tag="hm2")
            if _HM2_POOL:
                nc.gpsimd.tensor_sub(hm2[:], mh[:], one4[:])
            else:
                dmae.dma_start(hm2[:], hm2_d[i].rearrange("(j p) w -> p j w", p=128))
            # M^T: mtb[p, jw, jh*128+q] = M[jh*128+q, jw*128+p]
            mtb = ipool.tile([128, 4, _H], BF16, name="mtb", tag="mtb")
            if _HOST_MT:
                (nc.gpsimd if _GP_MT else nc.sync).dma_start(
                    mtb[:], maskt_d[i].rearrange("(j p) h -> p j h", p=128)
                )
            elif _DMA_T_MT:
                for jh in range(4):
                    nc.sync.dma_start_transpose(mtb[:, :, ts(jh, 128)], mb[:, jh, :])
            else:
                for jw in range(4):
                    mtp = put.tile([128, _W], BF16, name="mtp", tag="utp")
                    for jh in range(4):
                        nc.tensor.transpose(
                            mtp[:, ts(jh, 128)], mb[:, jh, ts(jw, 128)], ident_sb[:]
                        )
                    nc.vector.tensor_copy(mtb[:, jw, :], mtp[:])

            # ------------- box filter: U = B @ M^T (W direction) -------------
            ub = mpool.tile([128, 4, _H], BF16, name="ub", tag="ub")
            for iw in range(4):
                up = pup.tile([128, _H], F32, name="up", tag="up")
                js = [j for j in (iw - 1, iw, iw + 1) if 0 <= j < 4]
                for n, j in enumerate(js):
                    nc.tensor.matmul(
                        out=up[:],
                        lhsT=band_sb[:, j, ts(iw, 128)],
                        rhs=mtb[:, j, :],
                        start=(n == 0),
                        stop=(n == len(js) - 1),
                    )
                nc.any.tensor_copy(ub[:, iw, :], up[:])
            # ---------- transpose U back to (H, W) ----------
            utb = mpool.tile([128, 4, _W], BF16, name="utb", tag="utb")
            if _DMA_T_UT:
                for iw in range(4):
                    nc.sync.dma_start_transpose(utb[:, :, ts(iw, 128)], ub[:, iw, :])
            else:
                for ih in range(4):
                    utp = put.tile([128, _W], BF16, name="utp", tag="utp")
                    for iw in range(4):
                        nc.tensor.transpose(
                            utp[:, ts(iw, 128)], ub[:, iw, ts(ih, 128)], ident_sb[:]
                        )
                    nc.vector.tensor_copy(utb[:, ih, :], utp[:])
            # ------------- T2 = B @ U^T (H direction) -------------
            x4 = mpool.tile([128, 4, _W], BF16, name="x4", tag="x4")
            if _XWIDE:
                tpw = ptp.tile([128, 4, _W], F32, name="tpw", tag="tp")
                for ih in range(4):
                    js = [j for j in (ih - 1, ih, ih + 1) if 0 <= j < 4]
                    for n, j in enumerate(js):
                        nc.tensor.matmul(
                            out=tpw[:, ih, :],
                            lhsT=band_sb[:, j, ts(ih, 128)],
                            rhs=utb[:, j, :],
                            start=(n == 0),
                            stop=(n == len(js) - 1),
                        )
                nc.vector.scalar_tensor_tensor(
                    out=x4[:],
                    in0=tpw[:],
                    scalar=1.0 / 961.0,
                    in1=mb[:],
                    op0=ALU.mult,
                    op1=ALU.subtract,
                )
            else:
                for ih in range(4):
                    tp = ptp.tile([128, _W], F32, name="tp", tag="tp")
                    js = [j for j in (ih - 1, ih, ih + 1) if 0 <= j < 4]
                    for n, j in enumerate(js):
                        nc.tensor.matmul(
                            out=tp[:],
                            lhsT=band_sb[:, j, ts(ih, 128)],
                            rhs=utb[:, j, :],
                            start=(n == 0),
                            stop=(n == len(js) - 1),
                        )
                    if _X_FUSED:
                        nc.vector.scalar_tensor_tensor(
                            out=x4[:, ih, :],
                            in0=tp[:],
                            scalar=1.0 / 961.0,
                            in1=mb[:, ih, :],
                            op0=ALU.mult,
                            op1=ALU.subtract,
                        )
                    else:
                        nc.any.tensor_scalar(
                            out=x4[:, ih, :],
                            in0=tp[:],
                            scalar1=1.0 / 961.0,
                            scalar2=None,
                            op0=ALU.mult,
                        )
                if not _X_FUSED:
                    nc.vector.tensor_sub(x4[:], x4[:], mb[:])
            # a = |x| with running per-partition sum(a)
            a4 = mpool.tile([128, 4, _W], BF16, name="a4", tag="a4")
            if _ABS_ACT:
                nc.scalar.activation(
                    a4[:], x4[:], AF.Abs, accum_out=acc[:, 2 * i + 1 : 2 * i + 2]
                )
            else:
                nc.vector.tensor_scalar(
                    out=a4[:],
                    in0=x4[:],
                    scalar1=0.0,
                    scalar2=None,
                    op0=ALU.abs_max,
                    accum_out=acc[:, 2 * i + 1 : 2 * i + 2],
                )

            # ---------------- pointwise path (bf16, all 2x TT) ----------------
            e4 = mpool.tile([128, 4, _W], BF16, name="e4", tag="e4")
            nc.scalar.activation(e4[:], pb[:], AF.Exp)
            sp4 = mpool.tile([128, 4, _W], BF16, name="sp4", tag="sp4")
            nc.scalar.activation(sp4[:], e4[:], AF.Ln, bias=1.0)
            den3 = mpool.tile([128, 4, _W], BF16, name="den3", tag="den3")
            nc.vector.tensor_add(den3[:], e4[:], mh[:])
            lnd = mpool.tile([128, 4, _W], BF16, name="lnd", tag="lnd")
            nc.scalar.activation(lnd[:], den3[:], AF.Ln)
            hen2 = mpool.tile([128, 4, _W], BF16, name="hen2", tag="hen2")
            nc.vector.tensor_mul(hen2[:], hm2[:], e4[:])
            if _INPLACE:
                num2 = hen2
            else:
                num2 = mpool.tile([128, 4, _W], BF16, name="num2", tag="num2")
            nc.vector.tensor_sub(num2[:], mh[:], hen2[:])
            lnn = mpool.tile([128, 4, _W], BF16, name="lnn", tag="lnn")
            nc.scalar.activation(lnn[:], num2[:], AF.Ln, bias=-0.5)
            if _INPLACE:
                diff = lnn
            else:
                diff = mpool.tile([128, 4, _W], BF16, name="diff", tag="diff")
            nc.vector.tensor_sub(diff[:], lnn[:], lnd[:])
            ratio = mpool.tile([128, 4, _W], BF16, name="ratio", tag="ratio")
            nc.scalar.activation(ratio[:], diff[:], AF.Exp)
            pm = mpool.tile([128, 4, _W], BF16, name="pm", tag="pm")
            nc.vector.tensor_mul(pm[:], pb[:], mb[:])
            if _INPLACE:
                t1 = sp4
            else:
                t1 = mpool.tile([128, 4, _W], BF16, name="t1", tag="t1")
            nc.vector.tensor_sub(t1[:], sp4[:], pm[:])
            if _INPLACE:
                t4 = t1
            else:
                t4 = mpool.tile([128, 4, _W], BF16, name="t4", tag="t4")
            nc.vector.tensor_add(t4[:], t1[:], ratio[:])
            # sum((a + 0.2) * t) per partition
            w4 = mpool.tile([128, 4, _W], BF16, name="w4", tag="x4")
            nc.vector.scalar_tensor_tensor(
                out=w4[:], in0=a4[:], scalar=0.2, in1=t4[:],
                op0=ALU.add, op1=ALU.mult,
                accum_out=acc[:, 2 * i : 2 * i + 1],
            )

        # -------- final 128-partition reduction of the accumulators --------
        fin = pfin.tile([1, 2 * _BPC], F32, name="fin", tag="fin")
        nc.tensor.matmul(
            out=fin[:], lhsT=ones_sb[:], rhs=acc[:], start=True, stop=True
        )
        res = cpool.tile([1, 2 * _BPC], F32, name="res", tag="res")
        nc.scalar.copy(res[:], fin[:])
        nc.sync.dma_start(out_d[:], res[:])

    nc.compile()
    _CACHE["nc"] = nc
    return nc


def _prep_inputs(pred, mask):
    import ml_dtypes

    bf16 = ml_dtypes.bfloat16
    p = np.asarray(pred, np.float32).reshape(_B, _H, _W)
    m = np.asarray(mask, np.float32).reshape(_B, _H, _W)
    pb = np.ascontiguousarray(p.astype(bf16))
    mb = np.ascontiguousarray(m.astype(bf16))
    mbt = np.ascontiguousarray(mb.transpose(0, 2, 1))
    m32 = mb.astype(np.float32)
    mh = np.ascontiguousarray(((m32 + 1.0) * 0.5).astype(bf16))
    hm2 = np.ascontiguousarray(((m32 - 1.0) * 0.5).astype(bf16))
    return pb, mb, mbt, mh, hm2


def run_cores(pred, mask, trace=False, tmpdir=None):
    """Run the SPMD kernel; returns (list of per-core out arrays, BassKernelResults)."""
    import ml_dtypes
    from concourse.bass_utils import run_bass_kernel_spmd

    nc = _build()
    pb, mb, mbt, mh, hm2 = _prep_inputs(pred, mask)
    band = _band_np()
    ident = np.eye(128, dtype=np.float32).astype(ml_dtypes.bfloat16)
    ones = np.ones((128, 1), np.float32)
    sl = lambda a, c: a[c * _BPC : (c + 1) * _BPC]
    in_maps = [
        {
            "pred": sl(pb, c),
            "mask": sl(mb, c),
            "mh": sl(mh, c),
            **({"maskt": sl(mbt, c)} if _HOST_MT else {}),
            "hm2": sl(hm2, c),
            "band": band,
            "ident": ident,
            "ones": ones,
        }
        for c in range(_NC)
    ]
    kw = {}
    if trace:
        kw = dict(trace=True, trace_cores=[0], tmpdir=tmpdir)
    br = run_bass_kernel_spmd(nc, in_maps, list(range(_NC)), **kw)
    outs = [br.results[c]["out"].reshape(2 * _BPC) for c in range(_NC)]
    return outs, br


def finish(outs):
    losses = []
    for c in range(_NC):
        o = outs[c].astype(np.float64)
        for i in range(_BPC):
            swt = o[2 * i]
            sa = o[2 * i + 1]
            losses.append(5.0 * swt / (_HW + 5.0 * sa))
    return np.float32(np.mean(losses))


def kernel(pred, mask):
    outs, _ = run_cores(pred, mask)
    return finish(outs)
